# revision 53
# baseline (speedup 1.0000x reference)
"""Trainium2 Bass kernel for nn_CPCircuitLayer.

Math: with all_indices the full cartesian grid (s = n // H, h = n % H),
    out[b, s, h] = sum_r seq_emb[b,s,r] * hid_emb[b,h,r] * cp[r]
                 = (seq_emb[b] @ diag(cp) @ hid_emb[b].T)[s, h]
where seq_emb[b] = X_b @ seq_W.T  (X_b = hidden_states[b], contract H)
      hid_emb[b] = X_b.T @ hid_W.T                        (contract S)

Sharding: 8 cores = (batch b, seq half) pairs. Each core receives X_b
fully (the hid factor contracts over all of S) with rows rotated so its
own seq half comes first, plus a host-transposed copy of that half
(xt = X_b[half].T), and computes
    hid_embT = (hid_W*cp) @ X_b          [R, H]
    seq_embT = seq_W @ X_b[half].T       [R, S/2]
    out_half = seq_embT.T @ hid_embT     [S/2, H]
writing its [512, 1024] slice of the output.

All device data is FP16 (PSUM accumulation FP32; output upcast on
host): rel err ~6e-4, half the HBM bytes, 3x the PE rate of fp32.

Measured DMA behavior that shapes the schedule: the two HWDGE queues
share one SDMA pool (~410 GB/s aggregate, ~205 GB/s each while both
stream), each queue drains its dma_starts in FIFO order, and a
dma_start's completion semaphore lags its data by up to ~2 us under
HBM load — so FEW, LARGE transfers win, and the count per queue is
sized so transfer time, not completion handling, dominates. Each
queue carries 3 input DMAs; the tiny weight tensor rides the separate
GpSimd SWDGE queue. xt (seq factor) and x (hid factor) chunks are
interleaved across the queues so the PE always has work and the last
arrival gates only the short hid tail. Every DMA region is fully
contiguous in DRAM (the host packs per-chunk layouts; strided DRAM
patterns halve HBM throughput). PE warmup matmuls on a memset scratch
start right after the preamble so the HAM clock gate is at 2.4 GHz
before real work. The tail: hid PSUM->SBUF casts split across Vector
and Scalar-ACT, final matmuls through a 5-deep PSUM bank rotation,
out-copies alternating DVE/ACT, and the tail-idle Sync engine
dispatches all four output row DMAs.
"""

import numpy as np

B, S, H, R = 4, 1024, 1024, 32
N_CORES = 8
SH = S // 2   # seq rows per core
KT = S // 128  # k-tiles over the contraction dims
MT = SH // 128  # row tiles in this core's seq half

_compiled = {}


def _np_fallback(hidden_states, all_indices, seq_W, hid_W, cp_weight):
    seq_emb = np.einsum("bsh,rh->bsr", hidden_states, seq_W)
    hid_emb = np.einsum("bsh,rs->bhr", hidden_states, hid_W)
    s_idx = all_indices[:, 0].astype(np.int64)
    h_idx = all_indices[:, 1].astype(np.int64)
    g_seq = seq_emb[:, s_idx, :]
    g_hid = hid_emb[:, h_idx, :]
    out = np.einsum("bnr,bnr,r->bn", g_seq, g_hid, cp_weight[0])
    return out.reshape(B, S, H).astype(np.float32)


def _pm(a):
    """[t*128, w] -> [128, t*w] partition-major pack (one contiguous
    chunk: partition p holds row p of every k-tile, tiles adjacent)."""
    t = a.shape[0] // 128
    return np.ascontiguousarray(
        a.reshape(t, 128, a.shape[1]).transpose(1, 0, 2).reshape(128, -1))


def _wtile(w):
    """[K, R] -> [128, KT*R] tile layout, partition-contiguous."""
    return np.ascontiguousarray(
        w.reshape(KT, 128, R).transpose(1, 0, 2).reshape(128, KT * R))


def build_raw_program():
    import contextlib

    import concourse.bass as bass
    import concourse.mybir as mybir

    f16 = mybir.dt.float16
    f32 = mybir.dt.float32

    nc = bass.Bass("TRN2", target_bir_lowering=False, debug=False,
                   num_devices=N_CORES, enable_partition_id=False)

    # per-chunk contiguous partition-major packs (host-side _pm)
    x_a = nc.dram_tensor("x_a", [128, 4 * H], f16, kind="ExternalInput")
    x_c = nc.dram_tensor("x_c", [128, 4 * H], f16, kind="ExternalInput")
    xt_a = nc.dram_tensor("xt_a", [128, 4 * SH], f16, kind="ExternalInput")
    xt_b = nc.dram_tensor("xt_b", [128, 4 * SH], f16, kind="ExternalInput")
    w_d = nc.dram_tensor("w", [128, 2 * KT * R], f16, kind="ExternalInput")
    out_d = nc.dram_tensor("out", [SH, H], f16, kind="ExternalOutput")

    with contextlib.ExitStack() as _xs:
        E = _xs.enter_context
        w_t = E(nc.sbuf_tensor([128, 2 * KT * R], f16))  # [p, sw | hw]
        x_t = E(nc.sbuf_tensor([128, KT * H], f16))      # tiles k0..k7
        xt_t = E(nc.sbuf_tensor([128, KT * SH], f16))
        hid_sb = E(nc.sbuf_tensor([R, H], f16))
        seq_sb = E(nc.sbuf_tensor([R, SH], f16))
        o_sb = E(nc.sbuf_tensor([128, MT * H], f16))
        scr_sb = E(nc.sbuf_tensor([128, 512], f16))
        scr2_sb = E(nc.sbuf_tensor([128, R], f16))
        hid_ps = E(nc.psum_tensor([R, H], f32))        # 2 banks
        seq_ps = E(nc.psum_tensor([R, SH], f32))       # 1 bank
        o_ps = [E(nc.psum_tensor(f"o_ps{i}", [128, 512], f32))
                for i in range(5)]                     # 5 banks
        dma_sem = E(nc.semaphore("dma_sem"))
        w_sem = E(nc.semaphore("w_sem"))
        pe_sem = E(nc.semaphore("pe_sem"))
        dve_sem = E(nc.semaphore("dve_sem"))
        act_sem = E(nc.semaphore("act_sem"))
        gp_sem = E(nc.semaphore("gp_sem"))
        xa_sem = E(nc.semaphore("xa_sem"))
        xc_sem = E(nc.semaphore("xc_sem"))
        xta_sem = E(nc.semaphore("xta_sem"))
        xtb_sem = E(nc.semaphore("xtb_sem"))
        block = E(nc.Block(no_gpsimd_drain=True))

        sw = lambda k: w_t.ap()[:, k * R:(k + 1) * R]
        hw = lambda k: w_t.ap()[:, KT * R + k * R:KT * R + (k + 1) * R]
        xk = lambda k, n: x_t.ap()[:, k * H + n * 512:k * H + (n + 1) * 512]
        xtk = lambda k: xt_t.ap()[:, k * SH:(k + 1) * SH]
        ob = lambda m, n: o_sb.ap()[:, m * H + n * 512:m * H + (n + 1) * 512]

        # queue plan (FIFO per queue; ~180-205 GB/s each while both
        # stream): x leads (hid's 3.4us of matmuls spreads through the
        # stream and its PSUM casts finish mid-stream); xt follows, so
        # the only work gated by the stream tail is seq's short chain.
        # Few large DMAs: a dma_start's completion semaphore lags its
        # data by ~1-2us under HBM load and the lags stack per queue.
        #   sync:   x_a(k0-3) | xt_a(k0-3)       -> 1.5 MB
        #   scalar: w | x_c(k4-7) | xt_b(k4-7)   -> 1.63 MB
        # The HAM clock gate needs ~3.4us of CONTINUOUS PE activity to
        # lift the PE from 1.2 to 2.4 GHz and flips back after ~1.7us
        # of warm idle: one long warmup burst + bridge dummies.
        # pe order: 12 warmups; hid k0-3 (incs 1-8); hid k4-7 (9-16);
        #   bridges; seq k0-3 (17-20); seq k4-7 (21-24);
        #   finals j=0..7 (25-32)
        # dve: 1 = hid_c0, 2-5 = seq_c m-blocks (split so finals start
        #   after block 0), 6.. = out (m,0) copies
        # act: 1 = hid_c1, 2.. = out (m,1) copies
        # out rows: 0,1,2 on sync; 3 on scalar after its copies

        @block.sync
        def _(sync):
            sync.dma_start(out=x_t.ap()[:, 0:4 * H],
                           in_=x_a[:]).then_inc(xa_sem, 16)
            sync.dma_start(out=xt_t.ap()[:, 0:4 * SH],
                           in_=xt_a[:]).then_inc(xta_sem, 16)
            for m in (0, 1, 2):                 # rows 0-2 on the sync queue
                sync.wait_ge(dve_sem, 6 + m)    # (m,0) copy
                sync.wait_ge(act_sem, 2 + m)    # (m,1) copy
                sync.dma_start(
                    out=out_d[m * 128:(m + 1) * 128, :],
                    in_=o_sb.ap()[:, m * H:(m + 1) * H],
                ).then_inc(dma_sem, 16)
            sync.wait_ge(dma_sem, 64)

        @block.tensor
        def _(tensor):
            # HAM warmup on scratch, no input dependency: PE at 2.4 GHz
            # by the time real data lands
            tensor.wait_ge(gp_sem, 1)
            for _ in range(12):
                nc.tensor.matmul(o_ps[0].ap()[0:R, :], scr_sb.ap()[:, 0:R],
                                 scr_sb.ap(), start=True, stop=True)

            def hid(k, first, last):
                for n in range(2):
                    nc.tensor.matmul(
                        hid_ps.ap()[:, n * 512:(n + 1) * 512],
                        hw(k), xk(k, n),
                        start=first, stop=last,
                    ).then_inc(pe_sem, 1)

            def bridge(count):
                # warm-state HAM flips back to 1.2 GHz after only
                # ~1.7us idle: touch the array across data gaps
                # (writes garbage to o_ps0; w_t loaded early)
                for _ in range(count):
                    nc.tensor.matmul(o_ps[0].ap()[0:R, :],
                                     w_t.ap()[:, 0:R], w_t.ap()[:, 0:512],
                                     start=True, stop=True)

            tensor.wait_ge(w_sem, 16)
            tensor.wait_ge(xa_sem, 16)
            for k in (0, 1, 2, 3):
                hid(k, k == 0, False)
            tensor.wait_ge(xc_sem, 16)
            for k in (4, 5, 6, 7):
                hid(k, False, k == 7)
            bridge(2)
            tensor.wait_ge(xta_sem, 16)
            for k in range(4):
                nc.tensor.matmul(
                    seq_ps.ap(), sw(k), xtk(k),
                    start=(k == 0), stop=False,
                ).then_inc(pe_sem, 1)
            tensor.wait_ge(xtb_sem, 16)
            for k in range(4, 8):
                nc.tensor.matmul(
                    seq_ps.ap(), sw(k), xtk(k),
                    start=False, stop=(k == 7),
                ).then_inc(pe_sem, 1)

            tensor.wait_ge(dve_sem, 2)   # hid_c0 + seq_c block 0
            for j in range(2 * MT):
                m, n = divmod(j, 2)
                if j == 1:
                    tensor.wait_ge(act_sem, 1)   # hid_c1 (odd j only)
                if j >= 2 and j % 2 == 0:
                    tensor.wait_ge(dve_sem, 2 + m)   # seq_c block m
                if j >= 5:
                    # WAR on recycled PSUM bank (5-deep rotation)
                    prev = j - 5
                    if prev % 2 == 0:
                        tensor.wait_ge(dve_sem, 6 + prev // 2)
                    else:
                        tensor.wait_ge(act_sem, 2 + (prev - 1) // 2)
                nc.tensor.matmul(
                    o_ps[j % 5].ap(),
                    seq_sb.ap()[:, m * 128:(m + 1) * 128],
                    hid_sb.ap()[:, n * 512:(n + 1) * 512],
                    start=True, stop=True,
                ).then_inc(pe_sem, 1)

        @block.vector
        def _(vector):
            vector.wait_ge(pe_sem, 15)   # hid k7 n0 (last) done
            nc.vector.tensor_copy(
                hid_sb.ap()[:, 0:512],
                hid_ps.ap()[:, 0:512]).then_inc(dve_sem, 1)
            vector.wait_ge(pe_sem, 24)   # seq k7 (last) done
            for m in range(MT):          # seq copy split per m-block
                nc.vector.tensor_copy(
                    seq_sb.ap()[:, m * 128:(m + 1) * 128],
                    seq_ps.ap()[:, m * 128:(m + 1) * 128],
                ).then_inc(dve_sem, 1)
            for j in range(0, 2 * MT, 2):    # out (m,0) copies
                m, n = divmod(j, 2)
                vector.wait_ge(pe_sem, 24 + j + 1)
                nc.vector.tensor_copy(
                    ob(m, n), o_ps[j % 5].ap()).then_inc(dve_sem, 1)

        @block.scalar
        def _(scalar):
            # second HWDGE queue
            scalar.dma_start(out=w_t.ap(), in_=w_d[:]).then_inc(w_sem, 16)
            scalar.dma_start(out=x_t.ap()[:, 4 * H:8 * H],
                            in_=x_c[:]).then_inc(xc_sem, 16)
            scalar.dma_start(out=xt_t.ap()[:, 4 * SH:8 * SH],
                            in_=xt_b[:]).then_inc(xtb_sem, 16)
            # dummy copy to pull the lazy ACT table load off the critical path
            scalar.wait_ge(w_sem, 16)
            nc.scalar.copy(scr2_sb.ap(), w_t.ap()[:, 0:R])
            scalar.wait_ge(pe_sem, 16)   # hid k3 n1 (last) done
            nc.scalar.copy(
                hid_sb.ap()[:, 512:1024],
                hid_ps.ap()[:, 512:1024]).then_inc(act_sem, 1)
            for j in range(1, 2 * MT, 2):    # out (m,1) copies
                m, n = divmod(j, 2)
                scalar.wait_ge(pe_sem, 24 + j + 1)
                nc.scalar.copy(
                    ob(m, n), o_ps[j % 5].ap()).then_inc(act_sem, 1)
            # last row right here: scalar is done copying, sync may still
            # be draining rows 0-2 dispatches
            scalar.wait_ge(dve_sem, 9)       # (3,0) copy
            scalar.wait_ge(act_sem, 5)       # own (3,1) copy (sim-visible)
            scalar.dma_start(
                out=out_d[3 * 128:4 * 128, :],
                in_=o_sb.ap()[:, 3 * H:4 * H],
            ).then_inc(dma_sem, 16)

        @block.gpsimd
        def _(gpsimd):
            gpsimd.memset(scr_sb.ap(), 0.0).then_inc(gp_sem, 1)

    return nc


def _get_program():
    if "nc" not in _compiled:
        _compiled["nc"] = build_raw_program()
    return _compiled["nc"]


def _make_in_maps(hidden_states, seq_W, hid_W, cp_weight):
    swT = _wtile(np.ascontiguousarray(seq_W.T))                    # [128, 256]
    hwT_rows = np.ascontiguousarray((hid_W * cp_weight[0][:, None]).T)  # [S, R]
    # per-half row rotation: own seq half first (hid contraction over S is
    # order-invariant as long as x rows and hw rows permute together)
    w_rot = [
        np.concatenate([swT, _wtile(np.concatenate(
            [hwT_rows[half * SH:], hwT_rows[:half * SH]], axis=0))],
            axis=1).astype(np.float16)
        for half in range(2)
    ]
    in_maps = []
    for c in range(N_CORES):
        b, half = divmod(c, 2)
        xb = hidden_states[b]
        if half:
            xb = np.concatenate([xb[SH:], xb[:SH]], axis=0)
        xb = xb.astype(np.float16)
        xt = np.ascontiguousarray(xb[:SH, :].T)   # [H, SH]
        in_maps.append({
            "x_a": _pm(xb[0:512]),
            "x_c": _pm(xb[512:1024]),
            "xt_a": _pm(xt[0:512]),
            "xt_b": _pm(xt[512:1024]),
            "w": w_rot[half],
        })
    return in_maps


def kernel(hidden_states, all_indices, seq_W, hid_W, cp_weight):
    hidden_states = np.asarray(hidden_states, dtype=np.float32)
    seq_W = np.asarray(seq_W, dtype=np.float32)
    hid_W = np.asarray(hid_W, dtype=np.float32)
    cp_weight = np.asarray(cp_weight, dtype=np.float32)
    idx = np.asarray(all_indices)

    # The reference's all_indices is always the full cartesian grid; verify
    # cheaply and fall back to a host path if ever not.
    n = np.arange(S * H, dtype=idx.dtype)
    if idx.shape != (S * H, 2) or not (
        np.array_equal(idx[:, 0], n // H) and np.array_equal(idx[:, 1], n % H)
    ):
        return _np_fallback(hidden_states, idx, seq_W, hid_W, cp_weight)

    from concourse.bass_utils import run_bass_kernel_spmd

    nc = _get_program()
    in_maps = _make_in_maps(hidden_states, seq_W, hid_W, cp_weight)
    res = run_bass_kernel_spmd(nc, in_maps, list(range(N_CORES)))

    out = np.empty((B, S, H), dtype=np.float32)
    for c in range(N_CORES):
        b, half = divmod(c, 2)
        out[b, half * SH:(half + 1) * SH, :] = (
            res.results[c]["out"].astype(np.float32))
    return out


# revision 59
# speedup vs baseline: 1.1599x; 1.1599x over previous
"""Trainium2 Bass kernel for nn_CPCircuitLayer.

Math: with all_indices the full cartesian grid (s = n // H, h = n % H),
    out[b, s, h] = sum_r seq_emb[b,s,r] * hid_emb[b,h,r] * cp[r]
                 = (seq_emb[b] @ diag(cp) @ hid_emb[b].T)[s, h]
where seq_emb[b] = X_b @ seq_W.T  (X_b = hidden_states[b], contract H)
      hid_emb[b] = X_b.T @ hid_W.T                        (contract S)

Sharding: 8 cores = (batch b, seq half) pairs. Each core receives X_b
fully (the hid factor contracts over all of S) with rows rotated so its
own seq half comes first, plus a host-transposed copy of that half
(xt = X_b[half].T), and computes
    hid_embT = (hid_W*cp) @ X_b          [R, H]
    seq_embT = seq_W @ X_b[half].T       [R, S/2]
    out_half = seq_embT.T @ hid_embT     [S/2, H]
writing its [512, 1024] slice of the output.

All device data is FP16 (PSUM accumulation FP32; output upcast on
host): rel err ~6e-4, half the HBM bytes, 3x the PE rate of fp32.

Measured DMA behavior that shapes the schedule: the two HWDGE queues
share one SDMA pool (~410 GB/s aggregate, ~205 GB/s each while both
stream), each queue drains its dma_starts in FIFO order, and a
dma_start's completion semaphore lags its data by up to ~2 us under
HBM load — so FEW, LARGE transfers win, and the count per queue is
sized so transfer time, not completion handling, dominates. Each
queue carries 3 input DMAs; the tiny weight tensor rides the separate
GpSimd SWDGE queue. xt (seq factor) and x (hid factor) chunks are
interleaved across the queues so the PE always has work and the last
arrival gates only the short hid tail. Every DMA region is fully
contiguous in DRAM (the host packs per-chunk layouts; strided DRAM
patterns halve HBM throughput). PE warmup matmuls on a memset scratch
start right after the preamble so the HAM clock gate is at 2.4 GHz
before real work. The tail: hid PSUM->SBUF casts split across Vector
and Scalar-ACT, final matmuls through a 5-deep PSUM bank rotation,
out-copies alternating DVE/ACT, and the tail-idle Sync engine
dispatches all four output row DMAs.
"""

import numpy as np

B, S, H, R = 4, 1024, 1024, 32
N_CORES = 8
SH = S // 2   # seq rows per core
KT = S // 128  # k-tiles over the contraction dims
MT = SH // 128  # row tiles in this core's seq half

_compiled = {}


def _np_fallback(hidden_states, all_indices, seq_W, hid_W, cp_weight):
    seq_emb = np.einsum("bsh,rh->bsr", hidden_states, seq_W)
    hid_emb = np.einsum("bsh,rs->bhr", hidden_states, hid_W)
    s_idx = all_indices[:, 0].astype(np.int64)
    h_idx = all_indices[:, 1].astype(np.int64)
    g_seq = seq_emb[:, s_idx, :]
    g_hid = hid_emb[:, h_idx, :]
    out = np.einsum("bnr,bnr,r->bn", g_seq, g_hid, cp_weight[0])
    return out.reshape(B, S, H).astype(np.float32)


def _pm(a):
    """[t*128, w] -> [128, t*w] partition-major pack (one contiguous
    chunk: partition p holds row p of every k-tile, tiles adjacent)."""
    t = a.shape[0] // 128
    return np.ascontiguousarray(
        a.reshape(t, 128, a.shape[1]).transpose(1, 0, 2).reshape(128, -1))


def _wtile(w):
    """[K, R] -> [128, KT*R] tile layout, partition-contiguous."""
    return np.ascontiguousarray(
        w.reshape(KT, 128, R).transpose(1, 0, 2).reshape(128, KT * R))


def build_raw_program():
    import contextlib

    import concourse.bass as bass
    import concourse.mybir as mybir

    f16 = mybir.dt.float16
    f32 = mybir.dt.float32

    nc = bass.Bass("TRN2", target_bir_lowering=False, debug=False,
                   num_devices=N_CORES, enable_partition_id=False)

    # per-chunk contiguous partition-major packs (host-side _pm)
    x_a = nc.dram_tensor("x_a", [128, 4 * H], f16, kind="ExternalInput")
    x_c = nc.dram_tensor("x_c", [128, 4 * H], f16, kind="ExternalInput")
    xt_a = nc.dram_tensor("xt_a", [128, 4 * SH], f16, kind="ExternalInput")
    xt_b = nc.dram_tensor("xt_b", [128, 4 * SH], f16, kind="ExternalInput")
    w_d = nc.dram_tensor("w", [128, 2 * KT * R], f16, kind="ExternalInput")
    out_d = nc.dram_tensor("out", [SH, H], f16, kind="ExternalOutput")

    with contextlib.ExitStack() as _xs:
        E = _xs.enter_context
        w_t = E(nc.sbuf_tensor([128, 2 * KT * R], f16))  # [p, sw | hw]
        x_t = E(nc.sbuf_tensor([128, KT * H], f16))      # tiles k0..k7
        xt_t = E(nc.sbuf_tensor([128, KT * SH], f16))
        hid_sb = E(nc.sbuf_tensor([R, H], f16))
        seq_sb = E(nc.sbuf_tensor([R, SH], f16))
        o_sb = E(nc.sbuf_tensor([128, MT * H], f16))
        scr_sb = E(nc.sbuf_tensor([128, 512], f16))
        scr2_sb = E(nc.sbuf_tensor([128, R], f16))
        hid_ps = E(nc.psum_tensor([R, H], f32))        # 2 banks
        seq_ps = E(nc.psum_tensor([R, SH], f32))       # 1 bank
        o_ps = [E(nc.psum_tensor(f"o_ps{i}", [128, 512], f32))
                for i in range(5)]                     # 5 banks
        dma_sem = E(nc.semaphore("dma_sem"))
        w_sem = E(nc.semaphore("w_sem"))
        pe_sem = E(nc.semaphore("pe_sem"))
        dve_sem = E(nc.semaphore("dve_sem"))
        act_sem = E(nc.semaphore("act_sem"))
        gp_sem = E(nc.semaphore("gp_sem"))
        xa_sem = E(nc.semaphore("xa_sem"))
        xc_sem = E(nc.semaphore("xc_sem"))
        xta_sem = E(nc.semaphore("xta_sem"))
        xtb_sem = E(nc.semaphore("xtb_sem"))
        block = E(nc.Block(no_gpsimd_drain=True))

        sw = lambda k: w_t.ap()[:, k * R:(k + 1) * R]
        hw = lambda k: w_t.ap()[:, KT * R + k * R:KT * R + (k + 1) * R]
        xk = lambda k, n: x_t.ap()[:, k * H + n * 512:k * H + (n + 1) * 512]
        xtk = lambda k: xt_t.ap()[:, k * SH:(k + 1) * SH]
        ob = lambda m, n: o_sb.ap()[:, m * H + n * 512:m * H + (n + 1) * 512]

        # queue plan (FIFO per queue; ~180-205 GB/s each while both
        # stream): x leads (hid's 3.4us of matmuls spreads through the
        # stream and its PSUM casts finish mid-stream); xt follows, so
        # the only work gated by the stream tail is seq's short chain.
        # Few large DMAs: a dma_start's completion semaphore lags its
        # data by ~1-2us under HBM load and the lags stack per queue.
        #   sync:   x_a(k0-3) | xt_a(k0-3)       -> 1.5 MB
        #   scalar: w | x_c(k4-7) | xt_b(k4-7)   -> 1.63 MB
        # The HAM clock gate needs ~3.4us of CONTINUOUS PE activity to
        # lift the PE from 1.2 to 2.4 GHz and flips back after ~1.7us
        # of warm idle: one long warmup burst + bridge dummies.
        # pe order: 12 warmups; hid k0-3 (incs 1-8); hid k4-7 (9-16);
        #   bridges; seq k0-3 (17-20); seq k4-7 (21-24);
        #   finals j=0..7 (25-32)
        # dve: 1 = hid_c0, 2-5 = seq_c m-blocks (split so finals start
        #   after block 0), 6.. = out (m,0) copies
        # act: 1 = hid_c1, 2.. = out (m,1) copies
        # out rows: 0,1,2 on sync; 3 on scalar after its copies

        @block.sync
        def _(sync):
            sync.dma_start(out=x_t.ap()[:, 0:4 * H],
                           in_=x_a[:]).then_inc(xa_sem, 16)
            sync.dma_start(out=xt_t.ap()[:, 0:4 * SH],
                           in_=xt_a[:]).then_inc(xta_sem, 16)
            for m in (0, 1, 2):                 # rows 0-2 on the sync queue
                sync.wait_ge(dve_sem, 6 + m)    # (m,0) copy
                sync.wait_ge(act_sem, 2 + m)    # (m,1) copy
                sync.dma_start(
                    out=out_d[m * 128:(m + 1) * 128, :],
                    in_=o_sb.ap()[:, m * H:(m + 1) * H],
                ).then_inc(dma_sem, 16)
            sync.wait_ge(dma_sem, 64)

        @block.tensor
        def _(tensor):
            # HAM warmup on scratch, no input dependency: PE at 2.4 GHz
            # by the time real data lands
            tensor.wait_ge(gp_sem, 1)
            for _ in range(12):
                nc.tensor.matmul(o_ps[0].ap()[0:R, :], scr_sb.ap()[:, 0:R],
                                 scr_sb.ap(), start=True, stop=True)

            def hid(k, first, last):
                for n in range(2):
                    nc.tensor.matmul(
                        hid_ps.ap()[:, n * 512:(n + 1) * 512],
                        hw(k), xk(k, n),
                        start=first, stop=last,
                    ).then_inc(pe_sem, 1)

            tensor.wait_ge(w_sem, 16)
            tensor.wait_ge(xa_sem, 16)
            for k in (0, 1, 2, 3):
                hid(k, k == 0, False)
            tensor.wait_ge(xc_sem, 16)
            for k in (4, 5, 6, 7):
                hid(k, False, k == 7)
            tensor.wait_ge(xta_sem, 16)
            for k in range(4):
                nc.tensor.matmul(
                    seq_ps.ap(), sw(k), xtk(k),
                    start=(k == 0), stop=False,
                ).then_inc(pe_sem, 1)
            tensor.wait_ge(xtb_sem, 16)
            for k in range(4, 8):
                nc.tensor.matmul(
                    seq_ps.ap(), sw(k), xtk(k),
                    start=False, stop=(k == 7),
                ).then_inc(pe_sem, 1)

            tensor.wait_ge(dve_sem, 2)   # hid_c0 + seq_c block 0
            for j in range(2 * MT):
                m, n = divmod(j, 2)
                if j == 1:
                    tensor.wait_ge(act_sem, 1)   # hid_c1 (odd j only)
                if j >= 2 and j % 2 == 0:
                    tensor.wait_ge(dve_sem, 2 + m)   # seq_c block m
                if j >= 5:
                    # WAR on recycled PSUM bank (5-deep rotation)
                    prev = j - 5
                    if prev % 2 == 0:
                        tensor.wait_ge(dve_sem, 6 + prev // 2)
                    else:
                        tensor.wait_ge(act_sem, 2 + (prev - 1) // 2)
                nc.tensor.matmul(
                    o_ps[j % 5].ap(),
                    seq_sb.ap()[:, m * 128:(m + 1) * 128],
                    hid_sb.ap()[:, n * 512:(n + 1) * 512],
                    start=True, stop=True,
                ).then_inc(pe_sem, 1)

        @block.vector
        def _(vector):
            vector.wait_ge(pe_sem, 15)   # hid k7 n0 (last) done
            nc.vector.tensor_copy(
                hid_sb.ap()[:, 0:512],
                hid_ps.ap()[:, 0:512]).then_inc(dve_sem, 1)
            vector.wait_ge(pe_sem, 24)   # seq k7 (last) done
            for m in range(MT):          # seq copy split per m-block
                nc.vector.tensor_copy(
                    seq_sb.ap()[:, m * 128:(m + 1) * 128],
                    seq_ps.ap()[:, m * 128:(m + 1) * 128],
                ).then_inc(dve_sem, 1)
            for j in range(0, 2 * MT, 2):    # out (m,0) copies
                m, n = divmod(j, 2)
                vector.wait_ge(pe_sem, 24 + j + 1)
                nc.vector.tensor_copy(
                    ob(m, n), o_ps[j % 5].ap()).then_inc(dve_sem, 1)

        @block.scalar
        def _(scalar):
            # second HWDGE queue
            scalar.dma_start(out=w_t.ap(), in_=w_d[:]).then_inc(w_sem, 16)
            scalar.dma_start(out=x_t.ap()[:, 4 * H:8 * H],
                            in_=x_c[:]).then_inc(xc_sem, 16)
            scalar.dma_start(out=xt_t.ap()[:, 4 * SH:8 * SH],
                            in_=xt_b[:]).then_inc(xtb_sem, 16)
            # dummy copy to pull the lazy ACT table load off the critical path
            scalar.wait_ge(w_sem, 16)
            nc.scalar.copy(scr2_sb.ap(), w_t.ap()[:, 0:R])
            scalar.wait_ge(pe_sem, 16)   # hid k3 n1 (last) done
            nc.scalar.copy(
                hid_sb.ap()[:, 512:1024],
                hid_ps.ap()[:, 512:1024]).then_inc(act_sem, 1)
            for j in range(1, 2 * MT, 2):    # out (m,1) copies
                m, n = divmod(j, 2)
                scalar.wait_ge(pe_sem, 24 + j + 1)
                nc.scalar.copy(
                    ob(m, n), o_ps[j % 5].ap()).then_inc(act_sem, 1)
            # last row right here: scalar is done copying, sync may still
            # be draining rows 0-2 dispatches
            scalar.wait_ge(dve_sem, 9)       # (3,0) copy
            scalar.wait_ge(act_sem, 5)       # own (3,1) copy (sim-visible)
            scalar.dma_start(
                out=out_d[3 * 128:4 * 128, :],
                in_=o_sb.ap()[:, 3 * H:4 * H],
            ).then_inc(dma_sem, 16)

        @block.gpsimd
        def _(gpsimd):
            gpsimd.memset(scr_sb.ap(), 0.0).then_inc(gp_sem, 1)

    return nc


def _get_program():
    if "nc" not in _compiled:
        _compiled["nc"] = build_raw_program()
    return _compiled["nc"]


def _make_in_maps(hidden_states, seq_W, hid_W, cp_weight):
    swT = _wtile(np.ascontiguousarray(seq_W.T))                    # [128, 256]
    hwT_rows = np.ascontiguousarray((hid_W * cp_weight[0][:, None]).T)  # [S, R]
    # per-half row rotation: own seq half first (hid contraction over S is
    # order-invariant as long as x rows and hw rows permute together)
    w_rot = [
        np.concatenate([swT, _wtile(np.concatenate(
            [hwT_rows[half * SH:], hwT_rows[:half * SH]], axis=0))],
            axis=1).astype(np.float16)
        for half in range(2)
    ]
    in_maps = []
    for c in range(N_CORES):
        b, half = divmod(c, 2)
        xb = hidden_states[b]
        if half:
            xb = np.concatenate([xb[SH:], xb[:SH]], axis=0)
        xb = xb.astype(np.float16)
        xt = np.ascontiguousarray(xb[:SH, :].T)   # [H, SH]
        in_maps.append({
            "x_a": _pm(xb[0:512]),
            "x_c": _pm(xb[512:1024]),
            "xt_a": _pm(xt[0:512]),
            "xt_b": _pm(xt[512:1024]),
            "w": w_rot[half],
        })
    return in_maps


def kernel(hidden_states, all_indices, seq_W, hid_W, cp_weight):
    hidden_states = np.asarray(hidden_states, dtype=np.float32)
    seq_W = np.asarray(seq_W, dtype=np.float32)
    hid_W = np.asarray(hid_W, dtype=np.float32)
    cp_weight = np.asarray(cp_weight, dtype=np.float32)
    idx = np.asarray(all_indices)

    # The reference's all_indices is always the full cartesian grid; verify
    # cheaply and fall back to a host path if ever not.
    n = np.arange(S * H, dtype=idx.dtype)
    if idx.shape != (S * H, 2) or not (
        np.array_equal(idx[:, 0], n // H) and np.array_equal(idx[:, 1], n % H)
    ):
        return _np_fallback(hidden_states, idx, seq_W, hid_W, cp_weight)

    from concourse.bass_utils import run_bass_kernel_spmd

    nc = _get_program()
    in_maps = _make_in_maps(hidden_states, seq_W, hid_W, cp_weight)
    res = run_bass_kernel_spmd(nc, in_maps, list(range(N_CORES)))

    out = np.empty((B, S, H), dtype=np.float32)
    for c in range(N_CORES):
        b, half = divmod(c, 2)
        out[b, half * SH:(half + 1) * SH, :] = (
            res.results[c]["out"].astype(np.float32))
    return out


# revision 60
# speedup vs baseline: 1.1610x; 1.0009x over previous
"""Trainium2 Bass kernel for nn_CPCircuitLayer.

Math: with all_indices the full cartesian grid (s = n // H, h = n % H),
    out[b, s, h] = sum_r seq_emb[b,s,r] * hid_emb[b,h,r] * cp[r]
                 = (seq_emb[b] @ diag(cp) @ hid_emb[b].T)[s, h]
where seq_emb[b] = X_b @ seq_W.T  (X_b = hidden_states[b], contract H)
      hid_emb[b] = X_b.T @ hid_W.T                        (contract S)

Sharding: 8 cores = (batch b, seq half) pairs. Each core receives X_b
fully (the hid factor contracts over all of S) with rows rotated so its
own seq half comes first, plus a host-transposed copy of that half
(xt = X_b[half].T), and computes
    hid_embT = (hid_W*cp) @ X_b          [R, H]
    seq_embT = seq_W @ X_b[half].T       [R, S/2]
    out_half = seq_embT.T @ hid_embT     [S/2, H]
writing its [512, 1024] slice of the output.

All device data is FP16 (PSUM accumulation FP32; output upcast on
host): rel err ~6e-4, half the HBM bytes, 3x the PE rate of fp32.

Measured DMA behavior that shapes the schedule: the two HWDGE queues
share one SDMA pool (~410 GB/s aggregate, ~205 GB/s each while both
stream), each queue drains its dma_starts in FIFO order, and a
dma_start's completion semaphore lags its data by up to ~2 us under
HBM load — so FEW, LARGE transfers win, and the count per queue is
sized so transfer time, not completion handling, dominates. Each
queue carries 3 input DMAs; the tiny weight tensor rides the separate
GpSimd SWDGE queue. xt (seq factor) and x (hid factor) chunks are
interleaved across the queues so the PE always has work and the last
arrival gates only the short hid tail. Every DMA region is fully
contiguous in DRAM (the host packs per-chunk layouts; strided DRAM
patterns halve HBM throughput). PE warmup matmuls on a memset scratch
start right after the preamble so the HAM clock gate is at 2.4 GHz
before real work. The tail: hid PSUM->SBUF casts split across Vector
and Scalar-ACT, final matmuls through a 5-deep PSUM bank rotation,
out-copies alternating DVE/ACT, and the tail-idle Sync engine
dispatches all four output row DMAs.
"""

import numpy as np

B, S, H, R = 4, 1024, 1024, 32
N_CORES = 8
SH = S // 2   # seq rows per core
KT = S // 128  # k-tiles over the contraction dims
MT = SH // 128  # row tiles in this core's seq half

_compiled = {}


def _np_fallback(hidden_states, all_indices, seq_W, hid_W, cp_weight):
    seq_emb = np.einsum("bsh,rh->bsr", hidden_states, seq_W)
    hid_emb = np.einsum("bsh,rs->bhr", hidden_states, hid_W)
    s_idx = all_indices[:, 0].astype(np.int64)
    h_idx = all_indices[:, 1].astype(np.int64)
    g_seq = seq_emb[:, s_idx, :]
    g_hid = hid_emb[:, h_idx, :]
    out = np.einsum("bnr,bnr,r->bn", g_seq, g_hid, cp_weight[0])
    return out.reshape(B, S, H).astype(np.float32)


def _pm(a):
    """[t*128, w] -> [128, t*w] partition-major pack (one contiguous
    chunk: partition p holds row p of every k-tile, tiles adjacent)."""
    t = a.shape[0] // 128
    return np.ascontiguousarray(
        a.reshape(t, 128, a.shape[1]).transpose(1, 0, 2).reshape(128, -1))


def _wtile(w):
    """[K, R] -> [128, KT*R] tile layout, partition-contiguous."""
    return np.ascontiguousarray(
        w.reshape(KT, 128, R).transpose(1, 0, 2).reshape(128, KT * R))


def build_raw_program():
    import contextlib

    import concourse.bass as bass
    import concourse.mybir as mybir

    f16 = mybir.dt.float16
    f32 = mybir.dt.float32

    nc = bass.Bass("TRN2", target_bir_lowering=False, debug=False,
                   num_devices=N_CORES, enable_partition_id=False)

    # per-chunk contiguous partition-major packs (host-side _pm)
    x_a = nc.dram_tensor("x_a", [128, 4 * H], f16, kind="ExternalInput")
    x_c = nc.dram_tensor("x_c", [128, 4 * H], f16, kind="ExternalInput")
    xt_a = nc.dram_tensor("xt_a", [128, 4 * SH], f16, kind="ExternalInput")
    xt_b = nc.dram_tensor("xt_b", [128, 4 * SH], f16, kind="ExternalInput")
    w_d = nc.dram_tensor("w", [128, 2 * KT * R], f16, kind="ExternalInput")
    out_d = nc.dram_tensor("out", [SH, H], f16, kind="ExternalOutput")

    with contextlib.ExitStack() as _xs:
        E = _xs.enter_context
        w_t = E(nc.sbuf_tensor([128, 2 * KT * R], f16))  # [p, sw | hw]
        x_t = E(nc.sbuf_tensor([128, KT * H], f16))      # tiles k0..k7
        xt_t = E(nc.sbuf_tensor([128, KT * SH], f16))
        hid_sb = E(nc.sbuf_tensor([R, H], f16))
        seq_sb = E(nc.sbuf_tensor([R, SH], f16))
        o_sb = E(nc.sbuf_tensor([128, MT * H], f16))
        scr_sb = E(nc.sbuf_tensor([128, 512], f16))
        scr2_sb = E(nc.sbuf_tensor([128, R], f16))
        hid_ps = E(nc.psum_tensor([R, H], f32))        # 2 banks
        seq_ps = E(nc.psum_tensor([R, SH], f32))       # 1 bank
        o_ps = [E(nc.psum_tensor(f"o_ps{i}", [128, 512], f32))
                for i in range(5)]                     # 5 banks
        dma_sem = E(nc.semaphore("dma_sem"))
        w_sem = E(nc.semaphore("w_sem"))
        pe_sem = E(nc.semaphore("pe_sem"))
        dve_sem = E(nc.semaphore("dve_sem"))
        act_sem = E(nc.semaphore("act_sem"))
        gp_sem = E(nc.semaphore("gp_sem"))
        xa_sem = E(nc.semaphore("xa_sem"))
        xc_sem = E(nc.semaphore("xc_sem"))
        xta_sem = E(nc.semaphore("xta_sem"))
        xtb_sem = E(nc.semaphore("xtb_sem"))
        block = E(nc.Block(no_gpsimd_drain=True))

        sw = lambda k: w_t.ap()[:, k * R:(k + 1) * R]
        hw = lambda k: w_t.ap()[:, KT * R + k * R:KT * R + (k + 1) * R]
        xk = lambda k, n: x_t.ap()[:, k * H + n * 512:k * H + (n + 1) * 512]
        xtk = lambda k: xt_t.ap()[:, k * SH:(k + 1) * SH]
        ob = lambda m, n: o_sb.ap()[:, m * H + n * 512:m * H + (n + 1) * 512]

        # queue plan (FIFO per queue; ~180-205 GB/s each while both
        # stream): x leads (hid's 3.4us of matmuls spreads through the
        # stream and its PSUM casts finish mid-stream); xt follows, so
        # the only work gated by the stream tail is seq's short chain.
        # Few large DMAs: a dma_start's completion semaphore lags its
        # data by ~1-2us under HBM load and the lags stack per queue.
        #   sync:   x_a(k0-3) | xt_a(k0-3)       -> 1.5 MB
        #   scalar: w | x_c(k4-7) | xt_b(k4-7)   -> 1.63 MB
        # The HAM clock gate needs ~3.4us of CONTINUOUS PE activity to
        # lift the PE from 1.2 to 2.4 GHz and flips back after ~1.7us
        # of warm idle: one long warmup burst + bridge dummies.
        # pe order: 12 warmups; hid k0-3 (incs 1-8); hid k4-7 (9-16);
        #   bridges; seq k0-3 (17-20); seq k4-7 (21-24);
        #   finals j=0..7 (25-32)
        # dve: 1 = hid_c0, 2-5 = seq_c m-blocks (split so finals start
        #   after block 0), 6.. = out (m,0) copies
        # act: 1 = hid_c1, 2.. = out (m,1) copies
        # out rows: 0,1,2 on sync; 3 on scalar after its copies

        @block.sync
        def _(sync):
            sync.dma_start(out=x_t.ap()[:, 0:4 * H],
                           in_=x_a[:]).then_inc(xa_sem, 16)
            sync.dma_start(out=xt_t.ap()[:, 0:4 * SH],
                           in_=xt_a[:]).then_inc(xta_sem, 16)
            for m in (0, 1, 2):                 # rows 0-2 on the sync queue
                sync.wait_ge(dve_sem, 6 + m)    # (m,0) copy
                sync.wait_ge(act_sem, 2 + m)    # (m,1) copy
                sync.dma_start(
                    out=out_d[m * 128:(m + 1) * 128, :],
                    in_=o_sb.ap()[:, m * H:(m + 1) * H],
                    single_packet=True,
                ).then_inc(dma_sem, 16)
            sync.wait_ge(dma_sem, 64)

        @block.tensor
        def _(tensor):
            # HAM warmup on scratch, no input dependency: PE at 2.4 GHz
            # by the time real data lands
            tensor.wait_ge(gp_sem, 1)
            for _ in range(12):
                nc.tensor.matmul(o_ps[0].ap()[0:R, :], scr_sb.ap()[:, 0:R],
                                 scr_sb.ap(), start=True, stop=True)

            def hid(k, first, last):
                for n in range(2):
                    nc.tensor.matmul(
                        hid_ps.ap()[:, n * 512:(n + 1) * 512],
                        hw(k), xk(k, n),
                        start=first, stop=last,
                    ).then_inc(pe_sem, 1)

            tensor.wait_ge(w_sem, 16)
            tensor.wait_ge(xa_sem, 16)
            for k in (0, 1, 2, 3):
                hid(k, k == 0, False)
            tensor.wait_ge(xc_sem, 16)
            for k in (4, 5, 6, 7):
                hid(k, False, k == 7)
            tensor.wait_ge(xta_sem, 16)
            for k in range(4):
                nc.tensor.matmul(
                    seq_ps.ap(), sw(k), xtk(k),
                    start=(k == 0), stop=False,
                ).then_inc(pe_sem, 1)
            tensor.wait_ge(xtb_sem, 16)
            for k in range(4, 8):
                nc.tensor.matmul(
                    seq_ps.ap(), sw(k), xtk(k),
                    start=False, stop=(k == 7),
                ).then_inc(pe_sem, 1)

            tensor.wait_ge(dve_sem, 2)   # hid_c0 + seq_c block 0
            for j in range(2 * MT):
                m, n = divmod(j, 2)
                if j == 1:
                    tensor.wait_ge(act_sem, 1)   # hid_c1 (odd j only)
                if j >= 2 and j % 2 == 0:
                    tensor.wait_ge(dve_sem, 2 + m)   # seq_c block m
                if j >= 5:
                    # WAR on recycled PSUM bank (5-deep rotation)
                    prev = j - 5
                    if prev % 2 == 0:
                        tensor.wait_ge(dve_sem, 6 + prev // 2)
                    else:
                        tensor.wait_ge(act_sem, 2 + (prev - 1) // 2)
                nc.tensor.matmul(
                    o_ps[j % 5].ap(),
                    seq_sb.ap()[:, m * 128:(m + 1) * 128],
                    hid_sb.ap()[:, n * 512:(n + 1) * 512],
                    start=True, stop=True,
                ).then_inc(pe_sem, 1)

        @block.vector
        def _(vector):
            vector.wait_ge(pe_sem, 15)   # hid k7 n0 (last) done
            nc.vector.tensor_copy(
                hid_sb.ap()[:, 0:512],
                hid_ps.ap()[:, 0:512]).then_inc(dve_sem, 1)
            vector.wait_ge(pe_sem, 24)   # seq k7 (last) done
            for m in range(MT):          # seq copy split per m-block
                nc.vector.tensor_copy(
                    seq_sb.ap()[:, m * 128:(m + 1) * 128],
                    seq_ps.ap()[:, m * 128:(m + 1) * 128],
                ).then_inc(dve_sem, 1)
            for j in range(0, 2 * MT, 2):    # out (m,0) copies
                m, n = divmod(j, 2)
                vector.wait_ge(pe_sem, 24 + j + 1)
                nc.vector.tensor_copy(
                    ob(m, n), o_ps[j % 5].ap()).then_inc(dve_sem, 1)

        @block.scalar
        def _(scalar):
            # second HWDGE queue
            scalar.dma_start(out=w_t.ap(), in_=w_d[:]).then_inc(w_sem, 16)
            scalar.dma_start(out=x_t.ap()[:, 4 * H:8 * H],
                            in_=x_c[:]).then_inc(xc_sem, 16)
            scalar.dma_start(out=xt_t.ap()[:, 4 * SH:8 * SH],
                            in_=xt_b[:]).then_inc(xtb_sem, 16)
            # dummy copy to pull the lazy ACT table load off the critical path
            scalar.wait_ge(w_sem, 16)
            nc.scalar.copy(scr2_sb.ap(), w_t.ap()[:, 0:R])
            scalar.wait_ge(pe_sem, 16)   # hid k3 n1 (last) done
            nc.scalar.copy(
                hid_sb.ap()[:, 512:1024],
                hid_ps.ap()[:, 512:1024]).then_inc(act_sem, 1)
            for j in range(1, 2 * MT, 2):    # out (m,1) copies
                m, n = divmod(j, 2)
                scalar.wait_ge(pe_sem, 24 + j + 1)
                nc.scalar.copy(
                    ob(m, n), o_ps[j % 5].ap()).then_inc(act_sem, 1)
            # last row right here: scalar is done copying, sync may still
            # be draining rows 0-2 dispatches
            scalar.wait_ge(dve_sem, 9)       # (3,0) copy
            scalar.wait_ge(act_sem, 5)       # own (3,1) copy (sim-visible)
            scalar.dma_start(
                out=out_d[3 * 128:4 * 128, :],
                in_=o_sb.ap()[:, 3 * H:4 * H],
                single_packet=True,
            ).then_inc(dma_sem, 16)

        @block.gpsimd
        def _(gpsimd):
            gpsimd.memset(scr_sb.ap(), 0.0).then_inc(gp_sem, 1)

    return nc


def _get_program():
    if "nc" not in _compiled:
        _compiled["nc"] = build_raw_program()
    return _compiled["nc"]


def _make_in_maps(hidden_states, seq_W, hid_W, cp_weight):
    swT = _wtile(np.ascontiguousarray(seq_W.T))                    # [128, 256]
    hwT_rows = np.ascontiguousarray((hid_W * cp_weight[0][:, None]).T)  # [S, R]
    # per-half row rotation: own seq half first (hid contraction over S is
    # order-invariant as long as x rows and hw rows permute together)
    w_rot = [
        np.concatenate([swT, _wtile(np.concatenate(
            [hwT_rows[half * SH:], hwT_rows[:half * SH]], axis=0))],
            axis=1).astype(np.float16)
        for half in range(2)
    ]
    in_maps = []
    for c in range(N_CORES):
        b, half = divmod(c, 2)
        xb = hidden_states[b]
        if half:
            xb = np.concatenate([xb[SH:], xb[:SH]], axis=0)
        xb = xb.astype(np.float16)
        xt = np.ascontiguousarray(xb[:SH, :].T)   # [H, SH]
        in_maps.append({
            "x_a": _pm(xb[0:512]),
            "x_c": _pm(xb[512:1024]),
            "xt_a": _pm(xt[0:512]),
            "xt_b": _pm(xt[512:1024]),
            "w": w_rot[half],
        })
    return in_maps


def kernel(hidden_states, all_indices, seq_W, hid_W, cp_weight):
    hidden_states = np.asarray(hidden_states, dtype=np.float32)
    seq_W = np.asarray(seq_W, dtype=np.float32)
    hid_W = np.asarray(hid_W, dtype=np.float32)
    cp_weight = np.asarray(cp_weight, dtype=np.float32)
    idx = np.asarray(all_indices)

    # The reference's all_indices is always the full cartesian grid; verify
    # cheaply and fall back to a host path if ever not.
    n = np.arange(S * H, dtype=idx.dtype)
    if idx.shape != (S * H, 2) or not (
        np.array_equal(idx[:, 0], n // H) and np.array_equal(idx[:, 1], n % H)
    ):
        return _np_fallback(hidden_states, idx, seq_W, hid_W, cp_weight)

    from concourse.bass_utils import run_bass_kernel_spmd

    nc = _get_program()
    in_maps = _make_in_maps(hidden_states, seq_W, hid_W, cp_weight)
    res = run_bass_kernel_spmd(nc, in_maps, list(range(N_CORES)))

    out = np.empty((B, S, H), dtype=np.float32)
    for c in range(N_CORES):
        b, half = divmod(c, 2)
        out[b, half * SH:(half + 1) * SH, :] = (
            res.results[c]["out"].astype(np.float32))
    return out
